# revision 1
# baseline (speedup 1.0000x reference)
"""Trainium2 Bass kernel for nn_ColorsRasterizer.

Strategy (8 NeuronCores):
  data-parallel: core c -> mesh c//4, image rows [(c%4)*64, (c%4)*64+64).
  Each core rasterizes all F triangles against its 64x256 pixel strip.

Per core pipeline:
  1. preprocess (on device): gather verts/colors by faces (indirect DMA),
     project (x/z via exact reciprocal), build per-triangle affine
     coefficients for w0, w1, w2 and key = 4 - depth, and per-channel
     color-plane affine coefficients (records -> DRAM scratch).
  2. main loop over 128 pixel-groups (one row-half = 128 pixels on
     partitions): for each 512-triangle group, 4 row-packed rank-3 fp32
     matmuls evaluate w0/w1/w2/key for 128 pixels x 512 triangles; ACT
     turns negative w's into huge penalties (relu(-1e30*w), bf16); PE
     accumulates -penalties onto the key bank via identity matmuls; ACT
     copies the key bank into a [128, 14336] SBUF keys array.
     Winner per pixel = max_index (first occurrence => lowest triangle
     index, matching the reference tie-break).
  3. gather winning triangles' color-coefficient records, evaluate
     color = A*col + B*row + C per channel, apply coverage + positivity
     masks, DMA out.
"""
import os
import numpy as np

import concourse.bass as bass
import concourse.mybir as mybir
from concourse.tile import TileContext
from concourse.bass_utils import run_bass_kernel_spmd

f32 = mybir.dt.float32
bf16 = mybir.dt.bfloat16
i32 = mybir.dt.int32
u32 = mybir.dt.uint32
AL = mybir.AluOpType
AF = mybir.ActivationFunctionType

# problem constants (hardcoded per the task contract)
N, V, F, C = 2, 6890, 13776, 8
H, W = 256, 256
FP = 14336            # F padded to 128*112
TPP = FP // 128       # 112 triangles per partition in preproc layout
G_N, G_T = 28, 512    # main loop: 28 groups of 512 triangles
ROWS_PER_CORE = H // 4
PG64 = int(os.environ.get("KERNEL_PG64", "64"))  # rows per half in main loop
PEN_SCALE = -1.0e18   # relu(w * PEN_SCALE): huge but finite when w < 0


def _split_excess_waits(nc, max_waits=1):
    """This walrus build only accepts one sync-wait command per instruction;
    move extra waits onto preceding nops on the same engine."""
    for fn in nc.m.functions:
        for bb in fn.blocks:
            new = []
            changed = False
            for ins in bb.instructions:
                si = ins.sync_info
                w = list(si.on_wait) if (si and si.on_wait) else []
                if len(w) > max_waits:
                    changed = True
                    extra, keep = w[:-max_waits], w[-max_waits:]
                    for j, sw in enumerate(extra):
                        nop = mybir.InstNoOp(
                            name=f"{ins.name}-waitsplit{j}", engine=ins.engine)
                        nop.sync_info = mybir.SyncInfo(on_wait=[sw], on_update=[])
                        new.append(nop)
                    ins.sync_info = mybir.SyncInfo(
                        on_wait=keep,
                        on_update=list(si.on_update) if si.on_update else [])
                new.append(ins)
            if changed:
                bb.instructions = new


def _build_nc():
    nc = bass.Bass()

    vct_d = nc.dram_tensor("vct", [V, 12], f32, kind="ExternalInput")
    faces_d = nc.dram_tensor("facesp", [128, 3 * TPP], i32, kind="ExternalInput")
    btall_d = nc.dram_tensor("btall", [12, 128 * 128], f32, kind="ExternalInput")
    rowin_d = nc.dram_tensor("rowin", [128, 64], f32, kind="ExternalInput")
    colv_d = nc.dram_tensor("colv", [128, 2], f32, kind="ExternalInput")
    out_d = nc.dram_tensor("out", [128, 128 * C], f32, kind="ExternalOutput")
    DBG = os.environ.get("KERNEL_DEBUG_OUT", "0") == "1"
    if DBG:
        dkeys_d = nc.dram_tensor("dkeys", [128, FP], f32, kind="ExternalOutput")
        dwidx_d = nc.dram_tensor("dwidx", [128, 128], u32, kind="ExternalOutput")
        drmax_d = nc.dram_tensor("drmax", [128, 128], f32, kind="ExternalOutput")
        drh_d = nc.dram_tensor("drh", [128, FP], f32, kind="ExternalOutput")

    basis_rows = [0, 1, 2, 32, 33, 34, 64, 65, 66, 96, 97, 98]

    with TileContext(nc) as tc:
        with tc.tile_pool(name="dram", bufs=1, space="DRAM") as dpool, \
             tc.tile_pool(name="persist", bufs=1) as pp, \
             tc.tile_pool(name="rhpool", bufs=1) as rhpool:
            records = dpool.tile([FP, 24], f32)

            # ---- persistent SBUF ----
            BTALL = pp.tile([128, 128 * 128], f32)   # per-pixel-group matmul basis
            RIN = pp.tile([128, 64], f32)
            COLV = pp.tile([128, 2], f32)
            negI = pp.tile([128, 128], bf16)
            WIDX = pp.tile([128, 128], u32)    # winning triangle per pixel-group
            RMAX = pp.tile([128, 128], f32)    # winning key value
            RH = rhpool.tile([128, FP], f32)   # affine coefficients for matmul rhs

            for ki, rp_ in enumerate(basis_rows):
                nc.gpsimd.dma_start(BTALL[rp_:rp_ + 1, :], btall_d[ki:ki + 1, :])
            nc.gpsimd.dma_start(RIN[:, :], rowin_d[:, :])
            nc.gpsimd.dma_start(COLV[:, :], colv_d[:, :])

            nc.vector.memset(WIDX[:, :], 0)
            nc.vector.memset(RMAX[:, :], 0.0)
            nc.vector.memset(RH[:, :], 0.0)
            nc.gpsimd.memset(negI[:, :], -1.0)
            nc.gpsimd.affine_select(
                out=negI[:, :], in_=negI[:, :],
                compare_op=AL.is_equal, fill=0.0,
                base=0, pattern=[[-1, 128]], channel_multiplier=1)

            # ================= 1. preprocessing =================
            with tc.tile_pool(name="pre", bufs=1) as pr, \
                 tc.tile_pool(name="coef", bufs=1) as cf, \
                 tc.tile_pool(name="scr", bufs=6) as sc:
                FACES = pr.tile([128, 3 * TPP], i32)
                nc.gpsimd.dma_start(FACES[:, :], faces_d[:, :])

                # gather vertex+color rows: G_k[p, j*12 : j*12+12] = VCT[face[t,k]]
                GV = [pr.tile([128, 12 * TPP], f32, name=f"GV{k}") for k in range(3)]
                for k in range(3):
                    for j in range(TPP):
                        nc.gpsimd.indirect_dma_start(
                            out=GV[k][:, j * 12:(j + 1) * 12],
                            out_offset=None,
                            in_=vct_d[:, :],
                            in_offset=bass.IndirectOffsetOnAxis(
                                ap=FACES[:, j * 3 + k: j * 3 + k + 1], axis=0),
                        )

                def col(k, c):  # strided [128, TPP] view of component c of slot k
                    return GV[k][:, :].rearrange("p (j c) -> p j c", c=12)[:, :, c]

                def tmp(a, b, op):
                    o = sc.tile([128, TPP], f32, tag="tmp", name="tmp")
                    nc.vector.tensor_tensor(out=o[:, :], in0=a, in1=b, op=op)
                    return o

                def keep(name, a=None, b=None, op=None):
                    o = cf.tile([128, TPP], f32, tag=name, name=name)
                    if op is not None:
                        nc.vector.tensor_tensor(out=o[:, :], in0=a, in1=b, op=op)
                    return o

                # projection: px = x * (1/z), exact-rounded reciprocal
                px = [None] * 3
                py = [None] * 3
                for k in range(3):
                    rz = sc.tile([128, TPP], f32, tag="tmp", name="rz")
                    nc.vector.reciprocal(rz[:, :], col(k, 2))
                    px[k] = keep(f"px{k}", col(k, 0), rz[:, :], AL.mult)
                    py[k] = keep(f"py{k}", col(k, 1), rz[:, :], AL.mult)

                A0 = keep("A0", py[1][:, :], py[2][:, :], AL.subtract)
                B0 = keep("B0", px[2][:, :], px[1][:, :], AL.subtract)
                A1 = keep("A1", py[2][:, :], py[0][:, :], AL.subtract)
                B1 = keep("B1", px[0][:, :], px[2][:, :], AL.subtract)
                dy = tmp(py[0][:, :], py[2][:, :], AL.subtract)
                m1 = tmp(A0[:, :], B1[:, :], AL.mult)
                m2 = tmp(B0[:, :], dy[:, :], AL.mult)
                den = tmp(m1[:, :], m2[:, :], AL.add)

                absd = sc.tile([128, TPP], f32, tag="tmp", name="absd")
                nc.scalar.activation(absd[:, :], den[:, :], AF.Abs)
                degm = keep("degm")  # 1.0 if degenerate
                nc.vector.tensor_scalar(out=degm[:, :], in0=absd[:, :],
                                        scalar1=1e-8, scalar2=None, op0=AL.is_le)
                densafe = sc.tile([128, TPP], f32, tag="tmp", name="densafe")
                nc.vector.scalar_tensor_tensor(
                    out=densafe[:, :], in0=degm[:, :], scalar=1.0, in1=den[:, :],
                    op0=AL.mult, op1=AL.add)
                inv0 = sc.tile([128, TPP], f32, tag="tmp", name="inv0")
                nc.vector.reciprocal(inv0[:, :], densafe[:, :])
                nzm = keep("nzm")   # 1.0 if valid
                nc.vector.tensor_scalar(out=nzm[:, :], in0=degm[:, :],
                                        scalar1=-1.0, scalar2=1.0,
                                        op0=AL.mult, op1=AL.add)
                inv = keep("inv", inv0[:, :], nzm[:, :], AL.mult)
                ninv = keep("ninv")
                nc.vector.tensor_scalar(out=ninv[:, :], in0=inv[:, :],
                                        scalar1=-1.0, scalar2=None, op0=AL.mult)

                def affine_coeffs(Ak, Bk, nm):
                    a = keep(f"a{nm}", Ak[:, :], inv[:, :], AL.mult)
                    b = keep(f"b{nm}", Bk[:, :], inv[:, :], AL.mult)
                    t1 = tmp(Ak[:, :], px[2][:, :], AL.mult)
                    t2 = tmp(Bk[:, :], py[2][:, :], AL.mult)
                    s = tmp(t1[:, :], t2[:, :], AL.add)
                    c_ = keep(f"c{nm}", s[:, :], ninv[:, :], AL.mult)
                    return a, b, c_

                a0, b0, c0 = affine_coeffs(A0, B0, "w0")
                a1, b1, c1 = affine_coeffs(A1, B1, "w1")

                s01 = tmp(a0[:, :], a1[:, :], AL.add)
                a2 = keep("a2")
                nc.vector.tensor_scalar(out=a2[:, :], in0=s01[:, :],
                                        scalar1=-1.0, scalar2=None, op0=AL.mult)
                sb01 = tmp(b0[:, :], b1[:, :], AL.add)
                b2 = keep("b2")
                nc.vector.tensor_scalar(out=b2[:, :], in0=sb01[:, :],
                                        scalar1=-1.0, scalar2=None, op0=AL.mult)
                sc01 = tmp(c0[:, :], c1[:, :], AL.add)
                c2 = keep("c2")
                nc.vector.tensor_scalar(out=c2[:, :], in0=sc01[:, :],
                                        scalar1=-1.0, scalar2=1.0,
                                        op0=AL.mult, op1=AL.add)

                def zcombo(q0, q1, q2, nm):
                    # q0*z0 + q1*z1 + q2*z2
                    u0 = tmp(q0[:, :], col(0, 2), AL.mult)
                    u1 = tmp(q1[:, :], col(1, 2), AL.mult)
                    u2 = tmp(q2[:, :], col(2, 2), AL.mult)
                    s = tmp(u0[:, :], u1[:, :], AL.add)
                    return tmp(s[:, :], u2[:, :], AL.add)

                aD = zcombo(a0, a1, a2, "aD")
                bD = zcombo(b0, b1, b2, "bD")
                cD = zcombo(c0, c1, c2, "cD")
                adn = keep("adn")
                nc.vector.tensor_scalar(out=adn[:, :], in0=aD[:, :],
                                        scalar1=-1.0, scalar2=None, op0=AL.mult)
                bdn = keep("bdn")
                nc.vector.tensor_scalar(out=bdn[:, :], in0=bD[:, :],
                                        scalar1=-1.0, scalar2=None, op0=AL.mult)
                cdn = keep("cdn")
                nc.vector.tensor_scalar(out=cdn[:, :], in0=cD[:, :],
                                        scalar1=-1.0, scalar2=4.0,
                                        op0=AL.mult, op1=AL.add)

                # degenerate triangles: force w0 = -1e30 (=> key -inf)
                c0m = keep("c0m", c0[:, :], nzm[:, :], AL.mult)
                c0f = keep("c0f")
                nc.vector.scalar_tensor_tensor(
                    out=c0f[:, :], in0=degm[:, :], scalar=PEN_SCALE, in1=c0m[:, :],
                    op0=AL.mult, op1=AL.add)

                # fill RH rows: function q at partitions 32q..32q+2 (a, b, c)
                rows = {0: a0, 1: b0, 2: c0f,
                        32: a1, 33: b1, 34: c1,
                        64: a2, 65: b2, 66: c2,
                        96: adn, 97: bdn, 98: cdn}
                for r, tile_ in rows.items():
                    nc.gpsimd.dma_start(RH[r:r + 1, :], tile_[:, :])

                # color-plane records: rec[t] = (A_ch x8, B_ch x8, C_ch x8)
                REC = pr.tile([128, 24 * TPP], f32)
                REC3 = REC[:, :].rearrange("p (j m) -> p j m", m=24)
                for comp, (qa, qb, qc) in enumerate(((a0, a1, a2),
                                                     (b0, b1, b2),
                                                     (c0m, c1, c2))):
                    for ch in range(C):
                        v0 = tmp(qa[:, :], col(0, 3 + ch), AL.mult)
                        v1 = tmp(qb[:, :], col(1, 3 + ch), AL.mult)
                        v2 = tmp(qc[:, :], col(2, 3 + ch), AL.mult)
                        vs = tmp(v0[:, :], v1[:, :], AL.add)
                        nc.vector.tensor_tensor(
                            out=REC3[:, :, comp * 8 + ch],
                            in0=vs[:, :], in1=v2[:, :], op=AL.add)
                nc.gpsimd.dma_start(
                    records[:, :].rearrange("(p j) m -> p (j m)", p=128),
                    REC[:, :])

            # ================= 2. main loop =================
            with tc.tile_pool(name="keyspool", bufs=1) as kpool, \
                 tc.tile_pool(name="mainr", bufs=2) as rp, \
                 tc.tile_pool(name="mainx", bufs=2) as xp, \
                 tc.tile_pool(name="psum", bufs=2, space="PSUM") as psp:
                keys = kpool.tile([128, FP], f32)
                for h in range(2):
                    for r in range(PG64):
                        pg = h * 64 + r
                        bs = slice(pg * 128, (pg + 1) * 128)
                        for g in range(G_N):
                            PT = psp.tile([128, 4 * G_T], f32, tag="PT")
                            for q in range(4):
                                nc.tensor.matmul(
                                    PT[:, q * G_T:(q + 1) * G_T],
                                    BTALL[32 * q:32 * q + 3, bs],
                                    RH[32 * q:32 * q + 3, g * G_T:(g + 1) * G_T],
                                    start=True, stop=(q != 3),
                                    tile_position=(32 * q, 0),
                                    skip_group_check=True)
                            rr = []
                            for q in range(3):
                                rt = rp.tile([128, G_T], bf16, tag=f"r{q}",
                                             name=f"r{q}")
                                nc.scalar.activation(
                                    rt[:, :], PT[:, q * G_T:(q + 1) * G_T],
                                    AF.Relu, scale=PEN_SCALE)
                                rr.append(rt)
                            r01 = rp.tile([128, G_T], bf16, tag="r01", name="r01")
                            nc.vector.tensor_tensor(
                                out=r01[:, :], in0=rr[0][:, :], in1=rr[1][:, :],
                                op=AL.add)
                            nc.tensor.matmul(
                                PT[:, 3 * G_T:4 * G_T], negI[:, :], r01[:, :],
                                start=False, stop=False, skip_group_check=True)
                            nc.tensor.matmul(
                                PT[:, 3 * G_T:4 * G_T], negI[:, :], rr[2][:, :],
                                start=False, stop=True, skip_group_check=True)
                            nc.scalar.activation(
                                keys[:, g * G_T:(g + 1) * G_T],
                                PT[:, 3 * G_T:4 * G_T], AF.Copy)

                        top8 = xp.tile([128, 8], f32, tag="top8")
                        idx8 = xp.tile([128, 8], u32, tag="idx8")
                        nc.vector.max(top8[:, :], keys[:, :])
                        nc.vector.max_index(idx8[:, :], top8[:, :], keys[:, :])
                        nc.vector.tensor_copy(WIDX[:, pg:pg + 1], idx8[:, 0:1])
                        nc.vector.tensor_copy(RMAX[:, pg:pg + 1], top8[:, 0:1])
                        if DBG and pg == 0:
                            nc.gpsimd.dma_start(dkeys_d[:, :], keys[:, :])

            # ================= 3. winner gather + color eval =================
            if DBG:
                nc.gpsimd.dma_start(dwidx_d[:, :], WIDX[:, :])
                nc.gpsimd.dma_start(drmax_d[:, :], RMAX[:, :])
                nc.gpsimd.dma_start(drh_d[:, :], RH[:, :])
            with tc.tile_pool(name="fin", bufs=1) as fp_:
                RECG = fp_.tile([128, 128 * 24], f32)
                nc.vector.memset(RECG[:, :], 0.0)
                for pg in range(128):
                    nc.gpsimd.indirect_dma_start(
                        out=RECG[:, pg * 24:(pg + 1) * 24],
                        out_offset=None,
                        in_=records[:, :],
                        in_offset=bass.IndirectOffsetOnAxis(
                            ap=WIDX[:, pg:pg + 1], axis=0),
                    )

                RECG3 = RECG[:, :].rearrange("p (g m) -> p g m", m=24)
                OUT = fp_.tile([128, 128 * C], f32)
                OUT3 = OUT[:, :].rearrange("p (g c) -> p g c", c=C)
                for h in range(2):
                    gs = slice(h * 64, (h + 1) * 64)
                    T1 = fp_.tile([128, 64 * C], f32, tag="T1")
                    T13 = T1[:, :].rearrange("p (g c) -> p g c", c=C)
                    nc.vector.tensor_scalar(
                        out=T13[:, :, :], in0=RECG3[:, gs, 0:8],
                        scalar1=COLV[:, h:h + 1], scalar2=None, op0=AL.mult)
                    T2 = fp_.tile([128, 64 * C], f32, tag="T2")
                    T23 = T2[:, :].rearrange("p (g c) -> p g c", c=C)
                    rin3 = RIN[:, :].rearrange("p (g c) -> p g c", c=1)
                    nc.vector.tensor_tensor(
                        out=T23[:, :, :], in0=RECG3[:, gs, 8:16],
                        in1=rin3[:, :, :].to_broadcast([128, 64, C]), op=AL.mult)
                    nc.vector.tensor_tensor(
                        out=T13[:, :, :], in0=T13[:, :, :], in1=T23[:, :, :],
                        op=AL.add)
                    nc.vector.tensor_tensor(
                        out=OUT3[:, gs, :], in0=T13[:, :, :],
                        in1=RECG3[:, gs, 16:24], op=AL.add)

                # masks: covered (key > 1) AND any(channel > 0)
                GT = fp_.tile([128, 128 * C], f32)
                GT3 = GT[:, :].rearrange("p (g c) -> p g c", c=C)
                nc.vector.tensor_scalar(out=GT[:, :], in0=OUT[:, :],
                                        scalar1=0.0, scalar2=None, op0=AL.is_gt)
                CNT = fp_.tile([128, 128], f32)
                nc.vector.tensor_reduce(CNT[:, :], GT3[:, :, :],
                                        axis=mybir.AxisListType.X, op=AL.add)
                POS = fp_.tile([128, 128], f32)
                nc.vector.tensor_scalar(out=POS[:, :], in0=CNT[:, :],
                                        scalar1=0.0, scalar2=None, op0=AL.is_gt)
                COV = fp_.tile([128, 128], f32)
                nc.vector.tensor_scalar(out=COV[:, :], in0=RMAX[:, :],
                                        scalar1=1.0, scalar2=None, op0=AL.is_gt)
                MSK = fp_.tile([128, 128], f32)
                nc.vector.tensor_tensor(out=MSK[:, :], in0=POS[:, :],
                                        in1=COV[:, :], op=AL.mult)
                MSK3 = MSK[:, :].rearrange("p (g c) -> p g c", c=1)
                nc.vector.tensor_tensor(
                    out=OUT3[:, :, :], in0=OUT3[:, :, :],
                    in1=MSK3[:, :, :].to_broadcast([128, 128, C]), op=AL.mult)
                nc.gpsimd.dma_start(out_d[:, :], OUT[:, :])

    if os.environ.get("KERNEL_NO_WAITSPLIT", "0") != "1":
        _split_excess_waits(nc)
    return nc


_NC = None
LAST_RESULTS = None


def kernel(verts, colors, faces):
    global _NC, LAST_RESULTS
    verts = np.asarray(verts, dtype=np.float32)
    colors = np.asarray(colors, dtype=np.float32)
    faces = np.asarray(faces, dtype=np.int32)
    if _NC is None:
        _NC = _build_nc()
    nc = _NC

    faces_pad = np.zeros((FP, 3), np.int32)
    faces_pad[:F] = faces
    facesp = np.ascontiguousarray(faces_pad.reshape(128, TPP * 3))

    colv = np.zeros((128, 2), np.float32)
    colv[:, 0] = np.arange(128, dtype=np.float32)
    colv[:, 1] = np.arange(128, dtype=np.float32) + 128.0

    in_maps = []
    for c in range(8):
        m = c // 4
        rbase = (c % 4) * ROWS_PER_CORE
        vct = np.zeros((V, 12), np.float32)
        vct[:, 0:3] = verts[m]
        vct[:, 3:11] = colors[m]
        rowin = np.broadcast_to(
            rbase + np.arange(64, dtype=np.float32), (128, 64)).copy()
        btall = np.zeros((12, 128 * 128), np.float32)
        bt3 = btall.reshape(12, 128, 128)   # [k, pg, m]
        pgs = np.arange(128)
        hh, rr_ = pgs // 64, pgs % 64
        for q in range(4):
            bt3[3 * q + 0] = (hh * 128)[:, None] + np.arange(128)[None, :]
            bt3[3 * q + 1] = (rbase + rr_).astype(np.float32)[:, None]
            bt3[3 * q + 2] = 1.0
        in_maps.append({"vct": vct, "facesp": facesp, "btall": btall,
                        "rowin": rowin, "colv": colv})

    trace = bool(int(os.environ.get("KERNEL_TRACE", "0")))
    LAST_RESULTS = run_bass_kernel_spmd(
        nc, in_maps, core_ids=list(range(8)), trace=trace)

    full = np.zeros((N, C, H, W), np.float32)
    for c in range(8):
        m = c // 4
        rbase = (c % 4) * ROWS_PER_CORE
        arr = LAST_RESULTS.results[c]["out"].reshape(128, 2, 64, C)
        for h in range(2):
            # arr[p, h, r, ch] -> full[m, ch, rbase+r, h*128+p]
            full[m, :, rbase:rbase + 64, h * 128:(h + 1) * 128] = \
                np.transpose(arr[:, h], (2, 1, 0))
    return full

